# revision 1
# baseline (speedup 1.0000x reference)
"""CPC loss kernel for Trainium2 (Bass/Tile), data-parallel over batch on 8 NeuronCores.

Math: the reference computes, per forward step i = k+1 (k = 0..K-1):
    step_loss_k = -mean_{b, t in [0, T-i)} log(pos / neg)
with pos = exp(sum_e ce*be), neg = exp(sum_n sum_e ce*neg_n), so
    log(pos/neg) = sum_e ce[b,t,e] * (base[b,t+i,e] - negsum[b,e])
where ce = mask[b,t] * mapped_ctx[b,t,e,k] and negsum[b] = sum_n neg_samples[b,n].
The exp/log cancel exactly; the whole loss is a masked sum of dot products.

Device layout: e (=128) on partitions, t on the free dim, so the per-step shift
t -> t+k+1 is a free-dim offset. Per batch row, one 2-4MB DMA brings all K
mapped_ctx planes in. Per (row, k): one DVE multiply mctxT[e, t]*bmnT[e, t+k+1],
then a TensorE matmul whose stationary is a one-hot column (slice of a
precomputed "wide" matrix) reduces over partitions (e), landing the column sums
in PSUM partition r*K+k (zeros accumulate in the other partitions). A final
tensor_mul + reduce_sum applies the seq-len mask weights and reduces over t.
Host applies the per-step 1/(B*(T-i)) scaling.

Modes: "f32x" = all fp32 (exact, fp32 matmul at 1/4 rate);
       "f32"  = fp32 inputs, products rounded to float32r (full PE rate);
       "bf16" = bf16 inputs/products (halves DMA, 2x DVE).
"""

import numpy as np

B, T, E, K, NNEG = 64, 1024, 128, 8, 64
NCORES = 8
B_LOC = B // NCORES          # batch rows per core
TPAD = T + 8                 # bmn padded along t so every shifted read is in-bounds
L = T                        # compute width; t=T-1 column is always masked

MODE = "fp16"                # "f32x" | "f32" | "bf16" | "fp16"
_CACHE = {}
TRACE = False                # test harness may flip this for NTFF profiling
TRACE_KWARGS = {}
LAST_RESULTS = None


def _build(mode):
    from contextlib import ExitStack
    import concourse.bass as bass
    import concourse.bacc as bacc
    import concourse.tile as tile
    import concourse.mybir as mybir

    f32 = mybir.dt.float32
    cdt = {"f32x": f32, "f32": mybir.dt.float32r, "bf16": mybir.dt.bfloat16,
           "fp16": mybir.dt.float16}[mode]
    in_dt = cdt if mode in ("bf16", "fp16") else f32

    nc = bacc.Bacc(
        "TRN2",
        target_bir_lowering=False,
        debug=False,
        enable_asserts=False,
        num_devices=NCORES,
    )
    mctx_in = nc.dram_tensor("mctxT", [B_LOC, E, K, T], in_dt, kind="ExternalInput").ap()
    bmn_in = nc.dram_tensor("bmnT", [B_LOC, E, TPAD], in_dt, kind="ExternalInput").ap()
    w_in = nc.dram_tensor("w", [B_LOC * K, L], f32, kind="ExternalInput").ap()
    wide_in = nc.dram_tensor("wide", [E, 127], cdt, kind="ExternalInput").ap()
    s_out = nc.dram_tensor("S", [B_LOC * K, 1], f32, kind="ExternalOutput").ap()

    with tile.TileContext(nc) as tc, ExitStack() as ctx:
        m_pool = ctx.enter_context(tc.tile_pool(name="m", bufs=5))
        bmn_pool = ctx.enter_context(tc.tile_pool(name="bmn", bufs=3))
        prod_pool = ctx.enter_context(tc.tile_pool(name="prod", bufs=3))
        misc_pool = ctx.enter_context(tc.tile_pool(name="misc", bufs=1))
        psum_pool = ctx.enter_context(tc.tile_pool(name="psum", bufs=1, space="PSUM"))

        NR = B_LOC * K  # 64 psum rows, one per (r, k)
        # wide[:, 63] = 1, else 0. Slicing wide[:, 63-row : 127-row] gives a
        # [128, 64] one-hot-column stationary that lands the column sum of the
        # moving operand in PSUM partition `row` (zeros accumulate elsewhere).
        wide = misc_pool.tile([E, 127], cdt)
        nc.scalar.dma_start(wide[:], wide_in[:, :])
        wt = misc_pool.tile([NR, L], f32)
        nc.gpsimd.dma_start(wt[:], w_in[:, :])
        ps = psum_pool.tile([NR, L], f32)

        KH = K // 2
        for r in range(B_LOC):
            # two half-row DMAs on separate HWDGE queues (sync + scalar) so
            # they stream concurrently and the first muls start after ~1MB
            bmn = bmn_pool.tile([E, TPAD], in_dt)
            nc.sync.dma_start(bmn[:], bmn_in[r])
            m_lo = m_pool.tile([E, KH, T], in_dt, tag="m_lo")
            nc.sync.dma_start(m_lo[:], mctx_in[r, :, 0:KH, :])
            m_hi = m_pool.tile([E, KH, T], in_dt, tag="m_hi")
            nc.scalar.dma_start(m_hi[:], mctx_in[r, :, KH:K, :])
            if mode in ("bf16", "fp16"):
                # bmn_s[e, t] = bmn[e, t+1]: gives 4B-aligned window bases for
                # the even-k (odd-shift) fused multiply below.
                bmn_s = bmn_pool.tile([E, TPAD], in_dt, tag="bmn_s")
                nc.gpsimd.dma_start(bmn_s[:, 0:TPAD - 1], bmn[:, 1:TPAD])
            # Two fused multiplies per row, one per k-parity group (host lays
            # out planes in korder = [1,3,5,7,0,2,4,6]). The bmn operand is a
            # 3D AP of 4 overlapping shifted windows (k-dim step 2 elements).
            for half, m_half in (("lo", m_lo), ("hi", m_hi)):
                prod = prod_pool.tile([E, KH, T], cdt, tag=f"prod_{half}")
                if half == "lo":
                    # korig 1,3,5,7 -> shifts 2,4,6,8 (even, aligned)
                    src = bass.AP(bmn[:].tensor, 2, [[TPAD, E], [2, KH], [1, T]])
                elif mode in ("bf16", "fp16"):
                    # korig 0,2,4,6 -> shifts 1,3,5,7 via bmn_s at 0,2,4,6
                    src = bass.AP(bmn_s[:].tensor, 0, [[TPAD, E], [2, KH], [1, T]])
                else:
                    src = bass.AP(bmn[:].tensor, 1, [[TPAD, E], [2, KH], [1, T]])
                nc.vector.tensor_mul(prod[:, :, :], m_half[:, :, :], src)
                for j in range(KH):
                    row = r * K + (j if half == "lo" else KH + j)
                    oh = wide[:, NR - 1 - row:2 * NR - 1 - row]
                    first = row == 0
                    last = row == NR - 1
                    nc.tensor.matmul(
                        ps[:, 0:512], lhsT=oh,
                        rhs=prod[:, j, 0:512], start=first, stop=last,
                    )
                    nc.tensor.matmul(
                        ps[:, 512:L], lhsT=oh,
                        rhs=prod[:, j, 512:L], start=first, stop=last,
                    )

            if r == B_LOC // 2 - 1:
                # first-half finisher overlaps the remaining rows' compute
                scratch = misc_pool.tile([NR, L], f32)
                s_tile = misc_pool.tile([NR, 1], f32)
                half_rows = (B_LOC // 2) * K
                nc.vector.tensor_mul(
                    scratch[0:half_rows, :], ps[0:half_rows, :], wt[0:half_rows, :])
                nc.vector.reduce_sum(
                    s_tile[0:half_rows, :], scratch[0:half_rows, :],
                    axis=mybir.AxisListType.X)
                nc.scalar.dma_start(s_out[0:half_rows, :], s_tile[0:half_rows, :])

        nc.vector.tensor_mul(
            scratch[half_rows:NR, :], ps[half_rows:NR, :], wt[half_rows:NR, :])
        nc.vector.reduce_sum(
            s_tile[half_rows:NR, :], scratch[half_rows:NR, :],
            axis=mybir.AxisListType.X)
        nc.scalar.dma_start(s_out[half_rows:NR, :], s_tile[half_rows:NR, :])

    nc.compile()
    return nc


def kernel(base_emb, mapped_ctx, seq_lens, neg_ids):
    global LAST_RESULTS
    import ml_dtypes
    from concourse import bass_utils

    base = np.ascontiguousarray(np.asarray(base_emb, dtype=np.float32))
    mctx = np.asarray(mapped_ctx, dtype=np.float32)
    seq = np.asarray(seq_lens, dtype=np.int32)
    nids = np.asarray(neg_ids, dtype=np.int32)

    np_in_dt = {"bf16": ml_dtypes.bfloat16, "fp16": np.float16}.get(MODE, np.float32)

    # Host prep (sharding + per-batch-element negative gather, per sharding hint)
    neg_sum = base.reshape(B * T, E)[nids].sum(axis=1)             # [B, E]
    bmn = base - neg_sum[:, None, :]                               # [B, T, E]
    bmnT = np.zeros((B, E, TPAD), np_in_dt)
    bmnT[:, :, :T] = bmn.transpose(0, 2, 1)
    korder = [1, 3, 5, 7, 0, 2, 4, 6]
    mctxT = np.ascontiguousarray(
        mctx.transpose(0, 2, 3, 1)[:, :, korder, :].astype(np_in_dt))

    t_idx = np.arange(L)[None, None, :]                            # [1, 1, L]
    lim = np.minimum(seq[:, None], (T - 1 - np.arange(K))[None, :])  # [B, K]
    w = (t_idx < lim[:, :, None]).astype(np.float32)[:, korder, :]  # [B, K, L]
    wide = np.zeros((E, 127), np_in_dt if MODE in ("bf16", "fp16") else np.float32)
    wide[:, 63] = 1.0

    key = ("nc", MODE)
    if key not in _CACHE:
        _CACHE[key] = _build(MODE)
    nc = _CACHE[key]

    in_maps = []
    for c in range(NCORES):
        sl = slice(c * B_LOC, (c + 1) * B_LOC)
        in_maps.append({
            "mctxT": mctxT[sl],
            "bmnT": np.ascontiguousarray(bmnT[sl]),
            "w": np.ascontiguousarray(w[sl].reshape(B_LOC * K, L)),
            "wide": wide,
        })

    res = bass_utils.run_bass_kernel_spmd(
        nc, in_maps, core_ids=list(range(NCORES)), trace=TRACE, **TRACE_KWARGS
    )
    LAST_RESULTS = res

    S_dev = np.concatenate([r["S"].reshape(B_LOC, K) for r in res.results])  # [B, K(korder)]
    loss = 0.0
    for j, korig in enumerate(korder):
        loss += -S_dev[:, j].sum(dtype=np.float64) / (B * (T - korig - 1))
    loss /= K
    return np.float32(loss)

